# revision 2
# baseline (speedup 1.0000x reference)
"""DeepSeek-V3.1 MoE block (B=2,S=512,H=1024,I=512,E=64,topK=8) on 8 trn2 cores.

Strategy (expert-parallel, sparse dispatch, bf16):
  - The reference's dense-masked MoE is mathematically top-8 sparse: only the
    top-8 experts per token contribute. We exploit that.
  - Host: router in fp64 (selection margin ~4e-6 >> rounding noise), top-8 per
    token, per-expert token gather with capacity padding, everything cast to
    bf16 (end-to-end fro_rel ~4e-3, well under the 2e-2 gate).
  - Device, per core c (counts-sorted slot assignment): 8 experts/core.
    Transposed-intermediate SwiGLU: G^T/U^T = Wg/Wu^T-chunks (stationary)
    x X^T (moving) accumulate over H chunks -> PSUM [128i, tokens]; silu+mul
    on [128, IC*rows]; down-proj uses A^T directly as the stationary operand
    (no transposes at all); routing weight folded into the PSUM->SBUF copy of
    the output rows. Shared expert: token-parallel, 128 tokens/core.
  - Host: scatter-add per-expert bf16 outputs back by token, add shared.

  DMA-bound by weight streaming: ~27 MB bf16/core streamed once (~80 us at
  ~350 GB/s/core), PE ~60 us -> roofline ~85-100 us.
"""
import os as _os, sys
try:
    import concourse  # noqa: F401  (env-provided, e.g. axon boot path)
except ImportError:
    for _p in ('/root/.axon_site/_ro/trn_rl_repo', '/opt/trn_rl_repo'):
        if _os.path.isdir(_p) and _p not in sys.path:
            sys.path.append(_p)
import numpy as np
from ml_dtypes import bfloat16

B, S, H, I, E, TOPK = 2, 512, 1024, 512, 64, 8
T = B * S
NCORES = 8
ELOC = E // NCORES
HC, IC = H // 128, I // 128
TSH = T // NCORES  # shared-expert tokens per core (128)

LAST_RESULT = None  # BassKernelResults of the most recent run (for test harness)


def _pmajor(a, nchunk):
    """[nchunk*128, F] -> partition-major [128, nchunk*F] (chunk-row-major)."""
    F = a.shape[1]
    return np.ascontiguousarray(
        a.reshape(nchunk, 128, F).transpose(1, 0, 2).reshape(128, nchunk * F))


def _build(caps):
    import concourse.bacc as bacc
    import concourse.mybir as mybir
    from concourse import tile

    F32 = mybir.dt.float32
    BF16 = mybir.dt.bfloat16
    SILU = mybir.ActivationFunctionType.Silu

    # per-slot capacities (counts-sorted assignment): slot el holds capacity
    # caps[el]; flat tensors are concatenations over slots.
    xoff = np.concatenate([[0], np.cumsum([HC * c for c in caps])])
    yoff = np.concatenate([[0], np.cumsum(caps)])
    nb = [(c + 127) // 128 for c in caps]
    boff = np.concatenate([[0], np.cumsum(nb)])
    XW, YW, NBT = int(xoff[-1]), int(yoff[-1]), int(boff[-1])

    nc = bacc.Bacc("TRN2", target_bir_lowering=False, debug=False)

    xg_d = nc.dram_tensor("xg", [128, XW], BF16, kind="ExternalInput")
    wg_d = nc.dram_tensor("wg", [ELOC, 128, HC * I], BF16, kind="ExternalInput")
    wu_d = nc.dram_tensor("wu", [ELOC, 128, HC * I], BF16, kind="ExternalInput")
    wd_d = nc.dram_tensor("wd", [ELOC, 128, IC * H], BF16, kind="ExternalInput")
    cf_d = nc.dram_tensor("cf", [128, NBT], F32, kind="ExternalInput")
    xs_d = nc.dram_tensor("xs", [128, HC * TSH], BF16, kind="ExternalInput")
    wgs_d = nc.dram_tensor("wgs", [128, HC * I], BF16, kind="ExternalInput")
    wus_d = nc.dram_tensor("wus", [128, HC * I], BF16, kind="ExternalInput")
    wds_d = nc.dram_tensor("wds", [128, IC * H], BF16, kind="ExternalInput")
    yg_d = nc.dram_tensor("yg", [YW, H], BF16, kind="ExternalOutput")
    ys_d = nc.dram_tensor("ys", [TSH, H], BF16, kind="ExternalOutput")

    with tile.TileContext(nc) as tc:
        with (
            tc.tile_pool(name="const", bufs=1) as cpool,
            tc.tile_pool(name="wp", bufs=3) as wpool,
            tc.tile_pool(name="xp", bufs=3) as xpool,
            tc.tile_pool(name="ap", bufs=3) as apool,
            tc.tile_pool(name="ps", bufs=2, space="PSUM") as pspool,
        ):
            cf_all = cpool.tile([128, NBT], F32)
            nc.sync.dma_start(cf_all[:], cf_d[:])

            def ffn_block(xg_t, wg_t, wu_t, wd_t, rows, r0, C_in, coef_ap, out_ap):
                """One <=128-row token block through SwiGLU + down-proj.

                xg_t: [128, HC*C_in] X^T (bf16, partition-major over H);
                weights partition-major bf16; coef_ap [rows,1] f32 routing
                weight per token (or None); out_ap DRAM [rows,H] bf16.
                """
                w = IC * rows
                g_ps = pspool.tile([128, 512], F32, tag="g")
                u_ps = pspool.tile([128, 512], F32, tag="u")
                for t in range(IC):
                    for h in range(HC):
                        nc.tensor.matmul(
                            g_ps[:, t * rows:(t + 1) * rows],
                            wg_t[:, h * I + t * 128:h * I + (t + 1) * 128],
                            xg_t[:, h * C_in + r0:h * C_in + r0 + rows],
                            start=(h == 0), stop=(h == HC - 1))
                for t in range(IC):
                    for h in range(HC):
                        nc.tensor.matmul(
                            u_ps[:, t * rows:(t + 1) * rows],
                            wu_t[:, h * I + t * 128:h * I + (t + 1) * 128],
                            xg_t[:, h * C_in + r0:h * C_in + r0 + rows],
                            start=(h == 0), stop=(h == HC - 1))
                s_sb = apool.tile([128, 512], F32, tag="s")
                nc.scalar.activation(s_sb[:, :w], g_ps[:, :w], SILU)
                a_bf = apool.tile([128, 512], BF16, tag="a")
                nc.vector.tensor_mul(a_bf[:, :w], s_sb[:, :w], u_ps[:, :w])
                y_sb = apool.tile([128, H], BF16, tag="y")
                for half in range(2):
                    y_ps = pspool.tile([128, 512], F32, tag="y")
                    for t in range(IC):
                        nc.tensor.matmul(
                            y_ps[:rows],
                            a_bf[:, t * rows:(t + 1) * rows],
                            wd_t[:, t * H + 512 * half:t * H + 512 * (half + 1)],
                            start=(t == 0), stop=(t == IC - 1))
                    if coef_ap is not None:
                        nc.vector.tensor_scalar_mul(
                            y_sb[:rows, 512 * half:512 * (half + 1)],
                            y_ps[:rows], coef_ap)
                    else:
                        nc.vector.tensor_copy(
                            y_sb[:rows, 512 * half:512 * (half + 1)], y_ps[:rows])
                nc.gpsimd.dma_start(out_ap, y_sb[:rows])

            for e in range(ELOC):
                C = caps[e]
                blocks = [(r0, min(128, C - r0)) for r0 in range(0, C, 128)]
                wg_t = wpool.tile([128, HC * I], BF16, tag="wg")
                wu_t = wpool.tile([128, HC * I], BF16, tag="wu")
                wd_t = wpool.tile([128, IC * H], BF16, tag="wd")
                xg_t = xpool.tile([128, HC * max(caps)], BF16, tag="xg")
                nc.sync.dma_start(xg_t[:, :HC * C], xg_d[:, xoff[e]:xoff[e + 1]])
                hh = HC * I // 2
                nc.sync.dma_start(wg_t[:, :hh], wg_d[e][:, :hh])
                nc.sync.dma_start(wg_t[:, hh:], wg_d[e][:, hh:])
                nc.sync.dma_start(wu_t[:, :hh], wu_d[e][:, :hh])
                nc.sync.dma_start(wu_t[:, hh:], wu_d[e][:, hh:])
                ih = IC * H // 2
                nc.sync.dma_start(wd_t[:, :ih], wd_d[e][:, :ih])
                nc.sync.dma_start(wd_t[:, ih:], wd_d[e][:, ih:])
                for b, (r0, rows) in enumerate(blocks):
                    ffn_block(xg_t, wg_t, wu_t, wd_t, rows, r0, C,
                              cf_all[:rows, boff[e] + b:boff[e] + b + 1],
                              yg_d[yoff[e] + r0:yoff[e] + r0 + rows, :])

            # shared expert on this core's token slice
            wgs_t = wpool.tile([128, HC * I], BF16, tag="wg")
            wus_t = wpool.tile([128, HC * I], BF16, tag="wu")
            wds_t = wpool.tile([128, IC * H], BF16, tag="wd")
            xs_t = xpool.tile([128, HC * TSH], BF16, tag="xg")
            nc.sync.dma_start(xs_t[:], xs_d[:])
            nc.sync.dma_start(wgs_t[:], wgs_d[:])
            nc.sync.dma_start(wus_t[:], wus_d[:])
            nc.sync.dma_start(wds_t[:], wds_d[:])
            ffn_block(xs_t, wgs_t, wus_t, wds_t, TSH, 0, TSH, None, ys_d[:, :])

    nc.compile()
    return nc


def prepare(hidden_states, router_w, shared_gate_w, shared_up_w, shared_down_w,
            expert_gate_k, expert_up_k, expert_down_k):
    """Host-side routing + dispatch. Returns (nc, in_maps, meta)."""
    x = np.ascontiguousarray(np.asarray(hidden_states, dtype=np.float32).reshape(T, H))
    rw = np.asarray(router_w, dtype=np.float32)
    sgw = np.asarray(shared_gate_w, dtype=np.float32)
    suw = np.asarray(shared_up_w, dtype=np.float32)
    sdw = np.asarray(shared_down_w, dtype=np.float32)

    # ---- routing on host (fp64; selection margin >> fp32 noise) ----
    logits = x.astype(np.float64) @ rw.astype(np.float64)
    aff = 1.0 / (1.0 + np.exp(-logits))
    top_idx = np.argpartition(-aff, TOPK - 1, axis=1)[:, :TOPK]        # [T,8]
    top_vals = np.take_along_axis(aff, top_idx, axis=1)
    top_w = top_vals / (top_vals.sum(axis=1, keepdims=True) + 1e-9)    # [T,8]

    flat_e = top_idx.ravel()
    flat_t = np.repeat(np.arange(T), TOPK)
    flat_w = top_w.ravel()
    order = np.argsort(flat_e, kind="stable")
    se, st, sw = flat_e[order], flat_t[order], flat_w[order]
    counts = np.bincount(flat_e, minlength=E)
    offs = np.concatenate([[0], np.cumsum(counts)])

    # count-sorted assignment: slot el gets the el-th group of 8 heaviest
    # experts (one per core) -> light slots get smaller capacities.
    perm = np.argsort(-counts, kind="stable")          # experts by load desc
    slot_expert = perm.reshape(ELOC, NCORES)           # [slot, core] -> expert
    caps = [int(max(32, -(-counts[slot_expert[el]].max() // 32) * 32))
            for el in range(ELOC)]
    nb = [(c + 127) // 128 for c in caps]
    xoff = np.concatenate([[0], np.cumsum([HC * c for c in caps])])
    yoff = np.concatenate([[0], np.cumsum(caps)])
    boff = np.concatenate([[0], np.cumsum(nb)])

    nc = _build(caps)

    x16 = x.astype(bfloat16)
    egk16 = np.asarray(expert_gate_k, dtype=np.float32).astype(bfloat16)
    euk16 = np.asarray(expert_up_k, dtype=np.float32).astype(bfloat16)
    edk16 = np.asarray(expert_down_k, dtype=np.float32).astype(bfloat16)

    in_maps = []
    for c in range(NCORES):
        xg = np.zeros((128, int(xoff[-1])), bfloat16)
        cf = np.zeros((128, int(boff[-1])), np.float32)
        for el in range(ELOC):
            e = int(slot_expert[el, c])
            C = caps[el]
            toks = st[offs[e]:offs[e + 1]]
            ws = sw[offs[e]:offs[e + 1]]
            n = len(toks)
            xe = np.zeros((C, H), bfloat16)
            xe[:n] = x16[toks]
            xg[:, xoff[el]:xoff[el + 1]] = _pmajor(np.ascontiguousarray(xe.T), HC)
            cfp = np.zeros(nb[el] * 128, np.float32)
            cfp[:n] = ws
            cf[:, boff[el]:boff[el + 1]] = cfp.reshape(nb[el], 128).T
        def wstack(w, nchunk):  # [ELOC, nchunk*128, F] -> [ELOC, 128, nchunk*F]
            F = w.shape[2]
            return np.ascontiguousarray(
                w.reshape(ELOC, nchunk, 128, F).transpose(0, 2, 1, 3)
                 .reshape(ELOC, 128, nchunk * F))

        eids = slot_expert[:, c]
        in_maps.append({
            "xg": xg,
            "wg": wstack(np.ascontiguousarray(egk16[eids]), HC),
            "wu": wstack(np.ascontiguousarray(euk16[eids]), HC),
            "wd": wstack(np.ascontiguousarray(edk16[eids]), IC),
            "cf": cf,
            "xs": _pmajor(np.ascontiguousarray(x16[TSH * c:TSH * (c + 1)].T), HC),
            "wgs": _pmajor(sgw.astype(bfloat16), HC),
            "wus": _pmajor(suw.astype(bfloat16), HC),
            "wds": _pmajor(sdw.astype(bfloat16), IC),
        })

    return nc, in_maps, (st, offs, slot_expert, yoff)


def assemble(results, meta):
    st, offs, slot_expert, yoff = meta
    out = np.zeros((T, H), np.float32)
    for c in range(NCORES):
        r = results[c]
        out[TSH * c:TSH * (c + 1)] += np.asarray(r["ys"], np.float32)
        yg = np.asarray(r["yg"], np.float32)
        for el in range(ELOC):
            e = int(slot_expert[el, c])
            toks = st[offs[e]:offs[e + 1]]
            out[toks] += yg[yoff[el]:yoff[el] + len(toks)]
    return out.reshape(B, S, H)


def kernel(**inputs):
    global LAST_RESULT
    import os, time
    from concourse.bass_utils import run_bass_kernel_spmd
    if os.environ.get("BASS_TRACE"):
        try:
            import antenv.axon_hooks  # noqa: F401
        except ImportError:
            # trace requested but the axon NTFF hook module isn't present in
            # this container -- tracing would crash mid-run; disable it.
            os.environ["BASS_NEVER_TRACE"] = "1"
    nc, in_maps, meta = prepare(**inputs)
    last_err = None
    for attempt in range(3):
        try:
            res = run_bass_kernel_spmd(nc, in_maps, core_ids=list(range(NCORES)))
            break
        except Exception as err:  # transient device faults (e.g. NRT exec errors)
            last_err = err
            time.sleep(5 * (attempt + 1))
    else:
        raise last_err
    LAST_RESULT = res
    return assemble(res.results, meta)


# revision 7
# speedup vs baseline: 126.6569x; 126.6569x over previous
"""DeepSeek-V3.1 MoE block (B=2,S=512,H=1024,I=512,E=64,topK=8) on 8 trn2 cores.

Strategy (expert-parallel, sparse dispatch, bf16):
  - The reference's dense-masked MoE is mathematically top-8 sparse: only the
    top-8 experts per token contribute. We exploit that.
  - Host: router in fp64 (selection margin ~4e-6 >> rounding noise), top-8 per
    token, per-expert token gather with capacity padding, everything cast to
    bf16 (end-to-end fro_rel ~4e-3, well under the 2e-2 gate).
  - Device, per core c (counts-sorted slot assignment): 8 experts/core.
    Transposed-intermediate SwiGLU: G^T/U^T = Wg/Wu^T-chunks (stationary)
    x X^T (moving) accumulate over H chunks -> PSUM [128i, tokens]; silu+mul
    on [128, IC*rows]; down-proj uses A^T directly as the stationary operand
    (no transposes at all); routing weight folded into the PSUM->SBUF copy of
    the output rows. Shared expert: token-parallel, 128 tokens/core.
  - Host: scatter-add per-expert bf16 outputs back by token, add shared.

  DMA-bound by weight streaming: ~27 MB bf16/core streamed once (~80 us at
  ~350 GB/s/core), PE ~60 us -> roofline ~85-100 us.
"""
import os as _os, sys
try:
    import concourse  # noqa: F401  (env-provided, e.g. axon boot path)
except ImportError:
    for _p in ('/root/.axon_site/_ro/trn_rl_repo', '/opt/trn_rl_repo'):
        if _os.path.isdir(_p) and _p not in sys.path:
            sys.path.append(_p)
import numpy as np
from ml_dtypes import bfloat16

B, S, H, I, E, TOPK = 2, 512, 1024, 512, 64, 8
T = B * S
NCORES = 8
ELOC = E // NCORES
HC, IC = H // 128, I // 128
TSH = T // NCORES  # shared-expert tokens per core (128)

LAST_RESULT = None  # BassKernelResults of the most recent run (for test harness)


def _pmajor(a, nchunk):
    """[nchunk*128, F] -> partition-major [128, nchunk*F] (chunk-row-major)."""
    F = a.shape[1]
    return np.ascontiguousarray(
        a.reshape(nchunk, 128, F).transpose(1, 0, 2).reshape(128, nchunk * F))


def _build(caps, reps=1):
    import concourse.bacc as bacc
    import concourse.mybir as mybir
    from concourse import tile

    F32 = mybir.dt.float32
    BF16 = mybir.dt.bfloat16
    SILU = mybir.ActivationFunctionType.Silu

    # per-slot capacities (counts-sorted assignment): slot el holds capacity
    # caps[el]; flat tensors are concatenations over slots.
    xoff = np.concatenate([[0], np.cumsum([HC * c for c in caps])])
    yoff = np.concatenate([[0], np.cumsum(caps)])
    nb = [(c + 127) // 128 for c in caps]
    boff = np.concatenate([[0], np.cumsum(nb)])
    XW, YW, NBT = int(xoff[-1]), int(yoff[-1]), int(boff[-1])

    nc = bacc.Bacc("TRN2", target_bir_lowering=False, debug=False)

    xg_d = nc.dram_tensor("xg", [128, XW], BF16, kind="ExternalInput")
    wg_d = nc.dram_tensor("wg", [ELOC, 128, HC * I], BF16, kind="ExternalInput")
    wu_d = nc.dram_tensor("wu", [ELOC, 128, HC * I], BF16, kind="ExternalInput")
    wd_d = nc.dram_tensor("wd", [ELOC, 128, IC * H], BF16, kind="ExternalInput")
    cf_d = nc.dram_tensor("cf", [128, NBT], F32, kind="ExternalInput")
    xs_d = nc.dram_tensor("xs", [128, HC * TSH], BF16, kind="ExternalInput")
    wgs_d = nc.dram_tensor("wgs", [128, HC * I], BF16, kind="ExternalInput")
    wus_d = nc.dram_tensor("wus", [128, HC * I], BF16, kind="ExternalInput")
    wds_d = nc.dram_tensor("wds", [128, IC * H], BF16, kind="ExternalInput")
    yg_d = nc.dram_tensor("yg", [YW, H], BF16, kind="ExternalOutput")
    ys_d = nc.dram_tensor("ys", [TSH, H], BF16, kind="ExternalOutput")

    with tile.TileContext(nc) as tc:
        with (
            tc.tile_pool(name="const", bufs=1) as cpool,
            tc.tile_pool(name="wp", bufs=3) as wpool,
            tc.tile_pool(name="xp", bufs=3) as xpool,
            tc.tile_pool(name="ap", bufs=3) as apool,
            tc.tile_pool(name="ps", bufs=2, space="PSUM") as pspool,
        ):
            cf_all = cpool.tile([128, NBT], F32)
            nc.sync.dma_start(cf_all[:], cf_d[:])

            def ffn_block(xg_t, wg_t, wu_t, wd_t, rows, r0, C_in, coef_ap, out_ap):
                """One <=128-row token block through SwiGLU + down-proj.

                xg_t: [128, HC*C_in] X^T (bf16, partition-major over H);
                weights partition-major bf16; coef_ap [rows,1] f32 routing
                weight per token (or None); out_ap DRAM [rows,H] bf16.
                """
                w = IC * rows
                g_ps = pspool.tile([128, 512], F32, tag="g")
                u_ps = pspool.tile([128, 512], F32, tag="u")
                for t in range(IC):
                    for h in range(HC):
                        nc.tensor.matmul(
                            g_ps[:, t * rows:(t + 1) * rows],
                            wg_t[:, h * I + t * 128:h * I + (t + 1) * 128],
                            xg_t[:, h * C_in + r0:h * C_in + r0 + rows],
                            start=(h == 0), stop=(h == HC - 1))
                for t in range(IC):
                    for h in range(HC):
                        nc.tensor.matmul(
                            u_ps[:, t * rows:(t + 1) * rows],
                            wu_t[:, h * I + t * 128:h * I + (t + 1) * 128],
                            xg_t[:, h * C_in + r0:h * C_in + r0 + rows],
                            start=(h == 0), stop=(h == HC - 1))
                s_sb = apool.tile([128, 512], F32, tag="s")
                nc.scalar.activation(s_sb[:, :w], g_ps[:, :w], SILU)
                a_bf = apool.tile([128, 512], BF16, tag="a")
                nc.vector.tensor_mul(a_bf[:, :w], s_sb[:, :w], u_ps[:, :w])
                y_sb = apool.tile([128, H], BF16, tag="y")
                for half in range(2):
                    y_ps = pspool.tile([128, 512], F32, tag="y")
                    for t in range(IC):
                        nc.tensor.matmul(
                            y_ps[:rows],
                            a_bf[:, t * rows:(t + 1) * rows],
                            wd_t[:, t * H + 512 * half:t * H + 512 * (half + 1)],
                            start=(t == 0), stop=(t == IC - 1))
                    if coef_ap is not None:
                        nc.vector.tensor_scalar_mul(
                            y_sb[:rows, 512 * half:512 * (half + 1)],
                            y_ps[:rows], coef_ap)
                    else:
                        nc.vector.tensor_copy(
                            y_sb[:rows, 512 * half:512 * (half + 1)], y_ps[:rows])
                nc.gpsimd.dma_start(out_ap, y_sb[:rows])

            for _rep in range(reps):
                for e in range(ELOC):
                    C = caps[e]
                    blocks = [(r0, min(128, C - r0)) for r0 in range(0, C, 128)]
                    wg_t = wpool.tile([128, HC * I], BF16, tag="wg")
                    wu_t = wpool.tile([128, HC * I], BF16, tag="wu")
                    wd_t = wpool.tile([128, IC * H], BF16, tag="wd")
                    xg_t = xpool.tile([128, HC * max(caps)], BF16, tag="xg")
                    nc.sync.dma_start(xg_t[:, :HC * C], xg_d[:, xoff[e]:xoff[e + 1]])
                    hh = HC * I // 2
                    nc.sync.dma_start(wg_t[:, :hh], wg_d[e][:, :hh])
                    nc.sync.dma_start(wg_t[:, hh:], wg_d[e][:, hh:])
                    nc.sync.dma_start(wu_t[:, :hh], wu_d[e][:, :hh])
                    nc.sync.dma_start(wu_t[:, hh:], wu_d[e][:, hh:])
                    ih = IC * H // 2
                    nc.sync.dma_start(wd_t[:, :ih], wd_d[e][:, :ih])
                    nc.sync.dma_start(wd_t[:, ih:], wd_d[e][:, ih:])
                    for b, (r0, rows) in enumerate(blocks):
                        ffn_block(xg_t, wg_t, wu_t, wd_t, rows, r0, C,
                                  cf_all[:rows, boff[e] + b:boff[e] + b + 1],
                                  yg_d[yoff[e] + r0:yoff[e] + r0 + rows, :])

                # shared expert on this core's token slice
                wgs_t = wpool.tile([128, HC * I], BF16, tag="wg")
                wus_t = wpool.tile([128, HC * I], BF16, tag="wu")
                wds_t = wpool.tile([128, IC * H], BF16, tag="wd")
                xs_t = xpool.tile([128, HC * TSH], BF16, tag="xg")
                nc.sync.dma_start(xs_t[:], xs_d[:])
                nc.sync.dma_start(wgs_t[:], wgs_d[:])
                nc.sync.dma_start(wus_t[:], wus_d[:])
                nc.sync.dma_start(wds_t[:], wds_d[:])
                ffn_block(xs_t, wgs_t, wus_t, wds_t, TSH, 0, TSH, None, ys_d[:, :])

    nc.compile()
    return nc


def prepare(hidden_states, router_w, shared_gate_w, shared_up_w, shared_down_w,
            expert_gate_k, expert_up_k, expert_down_k, reps=1):
    """Host-side routing + dispatch. Returns (nc, in_maps, meta)."""
    x = np.ascontiguousarray(np.asarray(hidden_states, dtype=np.float32).reshape(T, H))
    rw = np.asarray(router_w, dtype=np.float32)
    sgw = np.asarray(shared_gate_w, dtype=np.float32)
    suw = np.asarray(shared_up_w, dtype=np.float32)
    sdw = np.asarray(shared_down_w, dtype=np.float32)

    # ---- routing on host (fp64; selection margin >> fp32 noise) ----
    logits = x.astype(np.float64) @ rw.astype(np.float64)
    aff = 1.0 / (1.0 + np.exp(-logits))
    top_idx = np.argpartition(-aff, TOPK - 1, axis=1)[:, :TOPK]        # [T,8]
    top_vals = np.take_along_axis(aff, top_idx, axis=1)
    top_w = top_vals / (top_vals.sum(axis=1, keepdims=True) + 1e-9)    # [T,8]

    flat_e = top_idx.ravel()
    flat_t = np.repeat(np.arange(T), TOPK)
    flat_w = top_w.ravel()
    order = np.argsort(flat_e, kind="stable")
    se, st, sw = flat_e[order], flat_t[order], flat_w[order]
    counts = np.bincount(flat_e, minlength=E)
    offs = np.concatenate([[0], np.cumsum(counts)])

    # count-sorted assignment: slot el gets the el-th group of 8 heaviest
    # experts (one per core) -> light slots get smaller capacities.
    perm = np.argsort(-counts, kind="stable")          # experts by load desc
    slot_expert = perm.reshape(ELOC, NCORES)           # [slot, core] -> expert
    caps = [int(max(32, -(-counts[slot_expert[el]].max() // 32) * 32))
            for el in range(ELOC)]
    nb = [(c + 127) // 128 for c in caps]
    xoff = np.concatenate([[0], np.cumsum([HC * c for c in caps])])
    yoff = np.concatenate([[0], np.cumsum(caps)])
    boff = np.concatenate([[0], np.cumsum(nb)])

    nc = _build(caps, reps=reps)

    x16 = x.astype(bfloat16)
    egk16 = np.asarray(expert_gate_k, dtype=np.float32).astype(bfloat16)
    euk16 = np.asarray(expert_up_k, dtype=np.float32).astype(bfloat16)
    edk16 = np.asarray(expert_down_k, dtype=np.float32).astype(bfloat16)

    in_maps = []
    for c in range(NCORES):
        xg = np.zeros((128, int(xoff[-1])), bfloat16)
        cf = np.zeros((128, int(boff[-1])), np.float32)
        for el in range(ELOC):
            e = int(slot_expert[el, c])
            C = caps[el]
            toks = st[offs[e]:offs[e + 1]]
            ws = sw[offs[e]:offs[e + 1]]
            n = len(toks)
            xe = np.zeros((C, H), bfloat16)
            xe[:n] = x16[toks]
            xg[:, xoff[el]:xoff[el + 1]] = _pmajor(np.ascontiguousarray(xe.T), HC)
            cfp = np.zeros(nb[el] * 128, np.float32)
            cfp[:n] = ws
            cf[:, boff[el]:boff[el + 1]] = cfp.reshape(nb[el], 128).T
        def wstack(w, nchunk):  # [ELOC, nchunk*128, F] -> [ELOC, 128, nchunk*F]
            F = w.shape[2]
            return np.ascontiguousarray(
                w.reshape(ELOC, nchunk, 128, F).transpose(0, 2, 1, 3)
                 .reshape(ELOC, 128, nchunk * F))

        eids = slot_expert[:, c]
        in_maps.append({
            "xg": xg,
            "wg": wstack(np.ascontiguousarray(egk16[eids]), HC),
            "wu": wstack(np.ascontiguousarray(euk16[eids]), HC),
            "wd": wstack(np.ascontiguousarray(edk16[eids]), IC),
            "cf": cf,
            "xs": _pmajor(np.ascontiguousarray(x16[TSH * c:TSH * (c + 1)].T), HC),
            "wgs": _pmajor(sgw.astype(bfloat16), HC),
            "wus": _pmajor(suw.astype(bfloat16), HC),
            "wds": _pmajor(sdw.astype(bfloat16), IC),
        })

    return nc, in_maps, (st, offs, slot_expert, yoff)


def assemble(results, meta):
    st, offs, slot_expert, yoff = meta
    out = np.zeros((T, H), np.float32)
    for c in range(NCORES):
        r = results[c]
        out[TSH * c:TSH * (c + 1)] += np.asarray(r["ys"], np.float32)
        yg = np.asarray(r["yg"], np.float32)
        for el in range(ELOC):
            e = int(slot_expert[el, c])
            toks = st[offs[e]:offs[e + 1]]
            out[toks] += yg[yoff[el]:yoff[el] + len(toks)]
    return out.reshape(B, S, H)


def kernel(**inputs):
    global LAST_RESULT
    import os, time
    from concourse.bass_utils import run_bass_kernel_spmd
    if os.environ.get("BASS_TRACE"):
        try:
            import antenv.axon_hooks  # noqa: F401
        except ImportError:
            # trace requested but the axon NTFF hook module isn't present in
            # this container -- tracing would crash mid-run; disable it.
            os.environ["BASS_NEVER_TRACE"] = "1"
    nc, in_maps, meta = prepare(**inputs)
    last_err = None
    for attempt in range(3):
        try:
            res = run_bass_kernel_spmd(nc, in_maps, core_ids=list(range(NCORES)))
            break
        except Exception as err:  # transient device faults (e.g. NRT exec errors)
            last_err = err
            time.sleep(5 * (attempt + 1))
    else:
        raise last_err
    LAST_RESULT = res
    return assemble(res.results, meta)
